# revision 1
# baseline (speedup 1.0000x reference)
"""Trainium2 Bass kernel for causal softmax-free multi-head attention (retention).

Reference computation (per batch b):
    kqv = x @ W1 + b1 ; k, q, v = split(kqv, 3)   [split order k, q, v]
    per head h (dh = 64):  attn = tril(q_h @ k_h^T) ; o_h = attn @ v_h
    out = concat_h(o_h) @ W2 + b2

Sharding: 8 cores = 2 batches x 4 head-groups (4 heads each). Each core
computes its batch's projections restricted to its heads' weight columns,
the attention for its 4 heads, and a partial output projection
(out_local @ W2[rows of its heads]). Host sums the 4 partials per batch.

bf16 build: all matmul operands bf16 (fp32 PSUM accumulation); measured
~114us HW exec (vs 147us fp32r baseline), rel err 6.8e-3 (gate 2e-2).
HW-validated facts this build relies on (see probe2.py):
  - bf16 matmuls allow K=64/M=64 and N=128 at full rate (1 col/cycle).
  - col-tiled pairs (tile_position=(0,64)) writing disjoint partition
    halves of one PSUM bank work, including later start=False accumulation
    over both halves.
  - diag tiles (64,64) with lhsT/rhs base partition 64 work.
  - row-tiled pairs writing the SAME bank crash the device -> scores for
    the 2 heads of a pair use zero-padded kT (K=128) into one bank.
  - PE transpose (bf16, via identity) works; used to derive k^T from the
    token-major kv tiles instead of recomputing k^T from x (saves 28k
    PE rows/core).

Algorithm: chunked linear attention, block C=128. Per head pair pg:
  scores  a_m = tril(Q_m K_m^T)           (1 bank, 4 matmuls/block)
  chains  o_m^T = V_m^T a_m + S^T-pass    (per-pg bank: h0 -> partitions
          0:64, h1 -> 64:128 via tile_position=(0,64); state pass via
          diag tiles using the block-diagonal quadrants of spad)
  state   S += K_m^T V_m                  (both heads in one matmul,
          diagonal quadrants accumulated into spad via DVE, bf16)
"""

import numpy as np
from ml_dtypes import bfloat16

import concourse.bacc as bacc
import concourse.mybir as mybir
import concourse.tile as tile
from concourse.bass_utils import run_bass_kernel_spmd

F32 = mybir.dt.float32
BF16 = mybir.dt.bfloat16
AF = mybir.ActivationFunctionType

B, T, D = 2, 2048, 1024
H, DH = 16, 64
HPC = 4             # heads per core
FH = HPC * DH       # 256 features per core per tensor
C = 128             # attention block size
NB = T // C         # 16 blocks
ND = D // 128       # 8 contraction chunks
NQT = T // 512      # 4 wide token tiles

TRACE = False
TRACE_DIR = None
LAST_RESULTS = [None]


def _build():
    nc = bacc.Bacc("TRN2", target_bir_lowering=False, debug=False, num_devices=8)

    xT = nc.dram_tensor("xT", [D, T], BF16, kind="ExternalInput").ap()
    w1q = nc.dram_tensor("w1q", [D, FH], BF16, kind="ExternalInput").ap()
    w1kv = nc.dram_tensor("w1kv", [D, 2 * FH], BF16, kind="ExternalInput").ap()
    b1q = nc.dram_tensor("b1q", [128, 2], F32, kind="ExternalInput").ap()
    bkv = nc.dram_tensor("bkv", [128, 512], BF16, kind="ExternalInput").ap()
    w2c = nc.dram_tensor("w2c", [128, 2 * D], BF16, kind="ExternalInput").ap()
    mask4 = nc.dram_tensor("mask4", [128, 512], BF16, kind="ExternalInput").ap()
    ident = nc.dram_tensor("ident", [128, 128], BF16, kind="ExternalInput").ap()
    out = nc.dram_tensor("out", [D, T], BF16, kind="ExternalOutput").ap()

    with tile.TileContext(nc) as tc:
        with (
            tc.tile_pool(name="persist", bufs=1) as pp,
            tc.tile_pool(name="work", bufs=3) as wp,
            tc.tile_pool(name="psA", bufs=4, space="PSUM") as psA,
            tc.tile_pool(name="psU", bufs=2, space="PSUM") as psU,
            tc.tile_pool(name="psO", bufs=2, space="PSUM") as psO,
        ):
            # ---- persistent SBUF tiles -------------------------------------
            w1q_sb = pp.tile([128, ND * FH], BF16, name="w1q_sb", tag="w1q_sb")
            w1kv_sb = pp.tile([128, ND * 2 * FH], BF16, name="w1kv_sb", tag="w1kv_sb")
            w2_sb = pp.tile([128, 2 * D], BF16, name="w2_sb", tag="w2_sb")
            b1q_sb = pp.tile([128, 2], F32, name="b1q_sb", tag="b1q_sb")
            bkv_sb = pp.tile([128, 512], BF16, name="bkv_sb", tag="bkv_sb")
            m4_sb = pp.tile([128, 512], BF16, name="m4_sb", tag="m4_sb")
            id_sb = pp.tile([128, 128], BF16, name="id_sb", tag="id_sb")
            xt = [pp.tile([128, T], BF16, name=f"xt{i}", tag=f"xt{i}") for i in range(ND)]
            qT = [pp.tile([128, T], BF16, name=f"qT{g}", tag=f"qT{g}") for g in range(2)]
            kTp = [pp.tile([128, T], BF16, name=f"kTp{h}", tag=f"kTp{h}") for h in range(4)]
            kv = [pp.tile([128, 512], BF16, name=f"kv{t}", tag=f"kv{t}") for t in range(NB)]
            oT = [pp.tile([128, T], BF16, name=f"oT{g}", tag=f"oT{g}") for g in range(2)]
            spad = [pp.tile([128, 128], BF16, name=f"spad{g}", tag=f"spad{g}") for g in range(2)]

            # ---- input DMAs ------------------------------------------------
            # sync queue: the critical q-projection stream, in consumption
            # order (w1q then x^T chunks). gpsimd queue: everything else.
            nc.sync.dma_start(
                out=w1q_sb.rearrange("p (c f) -> p c f", c=ND),
                in_=w1q.rearrange("(c p) f -> p c f", p=128))
            for i in range(ND):
                nc.sync.dma_start(out=xt[i][:], in_=xT[128 * i:128 * (i + 1), :])
            nc.gpsimd.dma_start(out=b1q_sb[:], in_=b1q[:])
            nc.gpsimd.dma_start(out=bkv_sb[:], in_=bkv[:])
            nc.gpsimd.dma_start(out=m4_sb[:], in_=mask4[:])
            nc.gpsimd.dma_start(out=id_sb[:], in_=ident[:])
            nc.gpsimd.dma_start(
                out=w1kv_sb.rearrange("p (c f) -> p c f", c=ND),
                in_=w1kv.rearrange("(c p) f -> p c f", p=128))
            nc.gpsimd.dma_start(out=w2_sb[:], in_=w2c[:])
            # zero fills: kTp pad halves + spad state (gpsimd memset, SBUF)
            for h in range(4):
                par = h % 2
                nc.gpsimd.memset(kTp[h][(1 - par) * 64:(2 - par) * 64, :], 0)
            for pg in range(2):
                nc.gpsimd.memset(spad[pg][:], 0)

            # ---- phase P: projections (waves of 8 PSUM groups) -------------
            _pools = [psA, psA, psA, psA, psU, psU, psO, psO]

            def run_wave(groups, pools=None):
                tiles = []
                for gi, _ in enumerate(groups):
                    pool = (pools or _pools)[gi]
                    tiles.append(pool.tile([128, 512], F32, name=f"pw{gi}",
                                           tag=["pa", "pu", "po"][[psA, psU, psO].index(pool)]))
                for d in range(ND):
                    for gi, (lf, rf, _) in enumerate(groups):
                        nc.tensor.matmul(
                            tiles[gi][:], lf(d), rf(d),
                            start=(d == 0), stop=(d == ND - 1))
                for gi, (_, _, cb) in enumerate(groups):
                    cb(tiles[gi])

            def q_group(ft, qt):
                def cb(pt):
                    nc.scalar.activation(
                        qT[ft][:, qt * 512:(qt + 1) * 512], pt[:],
                        AF.Identity, bias=b1q_sb[:, ft:ft + 1])
                return (
                    lambda d: w1q_sb[:, d * FH + ft * 128: d * FH + (ft + 1) * 128],
                    lambda d: xt[d][:, qt * 512:(qt + 1) * 512],
                    cb)

            def kv_group(tcn):
                def cb(pt):
                    nc.vector.tensor_tensor(
                        kv[tcn][:], pt[:], bkv_sb[:], mybir.AluOpType.add)
                return (
                    lambda d: xt[d][:, tcn * 128:(tcn + 1) * 128],
                    lambda d: w1kv_sb[:, d * 2 * FH:(d + 1) * 2 * FH],
                    cb)

            def transpose_k(tc_, on_act=False):
                # kv k-halves [128 tok, 128 kf(2 heads)] -> [128 kf, 128 tok]
                pt = psU.tile([128, 256], BF16, name="pt", tag="pu")
                sl = slice(tc_ * 128, (tc_ + 1) * 128)
                for pg in range(2):
                    nc.tensor.transpose(
                        pt[:, pg * 128:(pg + 1) * 128],
                        kv[tc_][:, pg * 128:(pg + 1) * 128], id_sb[:])
                for pg in range(2):
                    h0, h1 = 2 * pg, 2 * pg + 1
                    psl = slice(pg * 128, (pg + 1) * 128)
                    if on_act:
                        nc.scalar.activation(kTp[h0][0:64, sl], pt[0:64, psl], AF.Identity)
                        nc.scalar.activation(kTp[h1][64:128, sl], pt[64:128, psl], AF.Identity)
                    else:
                        nc.vector.tensor_copy(kTp[h0][0:64, sl], pt[0:64, psl])
                        nc.vector.tensor_copy(kTp[h1][64:128, sl], pt[64:128, psl])

            run_wave([q_group(0, 0), q_group(0, 1), q_group(0, 2), q_group(0, 3),
                      q_group(1, 0), q_group(1, 1), q_group(1, 2), q_group(1, 3)])
            run_wave([kv_group(t) for t in range(8)])
            for tc_ in range(8):
                transpose_k(tc_)
            run_wave([kv_group(t) for t in range(8, 16)])
            for tc_ in range(8, 12):
                transpose_k(tc_)

            # ---- phase A: chunked causal attention + interleaved D-proj ----
            ablk = {}

            pablk = {}

            def scores_mm(m):
                pa = psA.tile([128, 512], F32, name="pa", tag="pa")
                msl = slice(m * 128, (m + 1) * 128)
                for pg in range(2):
                    for par in range(2):
                        h = 2 * pg + par
                        nc.tensor.matmul(
                            pa[:, h * 128:(h + 1) * 128],
                            kTp[h][:, msl], qT[pg][:, msl],
                            start=True, stop=True)
                pablk[m] = pa

            def scores_mask(m):
                pa = pablk.pop(m)
                a = wp.tile([128, 512], BF16, name="a", tag="a", bufs=3)
                nc.vector.tensor_tensor(
                    a[:], pa[:], m4_sb[:], mybir.AluOpType.mult)
                ablk[m] = a

            def scores(m):
                scores_mm(m)
                scores_mask(m)

            def chains(m):
                msl = slice(m * 128, (m + 1) * 128)
                a = ablk.pop(m)
                for pg in range(2):
                    po = psO.tile([128, 512], F32, name="po", tag="po")
                    vbase = FH + pg * 128
                    abase = pg * 256
                    nc.tensor.matmul(
                        po[0:64, 0:128], kv[m][:, vbase:vbase + 64],
                        a[:, abase:abase + 128],
                        start=True, stop=True)
                    nc.tensor.matmul(
                        po[64:128, 0:128], kv[m][:, vbase + 64:vbase + 128],
                        a[:, abase + 128:abase + 256],
                        start=True, stop=True, tile_position=(0, 64))
                    if m > 0:
                        nc.tensor.matmul(
                            po[0:64, 0:128], spad[pg][0:64, 0:64],
                            qT[pg][0:64, msl],
                            start=False, stop=True, skip_group_check=True)
                        nc.tensor.matmul(
                            po[64:128, 0:128], spad[pg][64:128, 64:128],
                            qT[pg][64:128, msl],
                            start=False, stop=True, skip_group_check=True,
                            tile_position=(64, 64))
                    nc.scalar.activation(oT[pg][:, msl], po[:, 0:128], AF.Identity)

            pub = {}

            def state_update(m):
                pu = psU.tile([128, 512], F32, name="pu", tag="pu")
                for pg in range(2):
                    nc.tensor.matmul(
                        pu[:, pg * 128:(pg + 1) * 128],
                        kv[m][:, pg * 128:(pg + 1) * 128],
                        kv[m][:, FH + pg * 128:FH + (pg + 1) * 128],
                        start=True, stop=True)
                pub[m] = pu

            def state_add(m):
                pu = pub.pop(m)
                for pg in range(2):
                    nc.vector.tensor_tensor(
                        spad[pg][:], pu[:, pg * 128:(pg + 1) * 128], spad[pg][:],
                        mybir.AluOpType.add)

            def dproj(qt, pair):
                # 2 dout chunks, batched into one output DMA
                fs = wp.tile([128, 1024], BF16, name="fs", tag="fs", bufs=3)
                for i, dc in enumerate(range(pair * 2, pair * 2 + 2)):
                    pf = psA.tile([128, 512], F32, name="pf", tag="pa")
                    for pg in range(2):
                        nc.tensor.matmul(
                            pf[:],
                            w2_sb[:, pg * D + dc * 128: pg * D + (dc + 1) * 128],
                            oT[pg][:, qt * 512:(qt + 1) * 512],
                            start=(pg == 0), stop=(pg == 1))
                    if i % 2 == 0:
                        nc.vector.tensor_copy(fs[:, i * 512:(i + 1) * 512], pf[:])
                    else:
                        nc.scalar.activation(fs[:, i * 512:(i + 1) * 512], pf[:], AF.Identity)
                dma_eng = nc.gpsimd if (qt + pair) % 2 == 0 else nc.sync
                dma_eng.dma_start(
                    out=out[pair * 256:(pair + 1) * 256, qt * 512:(qt + 1) * 512]
                    .rearrange("(c p) f -> p c f", p=128),
                    in_=fs.rearrange("p (c f) -> p c f", c=2))

            def dproj256(qt, sub):
                # qt3 tail split: 8 dout chunks at 256-token granularity,
                # drained as two half-DMAs so the tail overlaps compute
                tsl = slice(qt * 512 + sub * 256, qt * 512 + (sub + 1) * 256)
                for hf in range(2):
                    fs = wp.tile([128, 1024], BF16, name="fs2", tag="fs2", bufs=2)
                    for i, dc in enumerate(range(hf * 4, hf * 4 + 4)):
                        pf = psA.tile([128, 512], F32, name="pf", tag="pa")
                        for pg in range(2):
                            nc.tensor.matmul(
                                pf[:, 0:256],
                                w2_sb[:, pg * D + dc * 128: pg * D + (dc + 1) * 128],
                                oT[pg][:, tsl],
                                start=(pg == 0), stop=(pg == 1))
                        if i % 2 == 0:
                            nc.vector.tensor_copy(fs[:, i * 256:(i + 1) * 256], pf[:, 0:256])
                        else:
                            nc.scalar.activation(fs[:, i * 256:(i + 1) * 256], pf[:, 0:256], AF.Identity)
                    dma_eng = nc.gpsimd if (sub + hf) % 2 == 0 else nc.sync
                    dma_eng.dma_start(
                        out=out[hf * 512:(hf + 1) * 512, tsl]
                        .rearrange("(c p) f -> p c f", p=128),
                        in_=fs.rearrange("p (c f) -> p c f", c=4))

            # software pipeline: scores(m+2) ahead of chains(m) so the DVE
            # mask is never on the PE critical path; pU emitted before chains
            # for extra independent PE work; D-proj spread at dc-pair
            # granularity once each qt's oT inputs (4 blocks) are complete.
            dplan = {m: (m // 4 - 1, m % 4) for m in range(4, 16)}
            scores(0)
            scores(1)
            for m in range(NB):
                if m + 2 < NB:
                    scores_mm(m + 2)
                if m < 4:
                    transpose_k(12 + m, on_act=True)
                if m < NB - 1:
                    state_update(m)
                chains(m)
                if m < NB - 1:
                    state_add(m)
                if m + 2 < NB:
                    scores_mask(m + 2)
                if m in dplan:
                    dproj(*dplan[m])
                if m == 14:
                    dproj256(3, 0)
            dproj256(3, 1)

    nc.compile()
    return nc


_NC = None


def _get_nc():
    global _NC
    if _NC is None:
        _NC = _build()
    return _NC


def make_core_inputs(x, W1, b1, W2, b2):
    """Shard full inputs into the 8 per-core input dicts (bf16)."""
    x = np.asarray(x, dtype=np.float32)
    W1 = np.asarray(W1, dtype=np.float32)
    b1 = np.asarray(b1, dtype=np.float32)
    W2 = np.asarray(W2, dtype=np.float32)

    p = np.arange(128)[:, None]
    f = np.arange(128)[None, :]
    tril = (f >= p).astype(np.float32)
    mask4 = np.concatenate([tril] * 4, axis=1).astype(bfloat16)
    ident = np.eye(128, dtype=np.float32).astype(bfloat16)

    xTb = [np.ascontiguousarray(x[b].T).astype(bfloat16) for b in range(B)]

    in_maps = []
    for c in range(8):
        b = c // 4
        g = c % 4
        ksl = slice(g * FH, (g + 1) * FH)
        qsl = slice(D + g * FH, D + (g + 1) * FH)
        vsl = slice(2 * D + g * FH, 2 * D + (g + 1) * FH)
        w2g = W2[ksl, :]                      # [256, 1024]
        w2c = np.ascontiguousarray(
            w2g.reshape(2, 128, D).transpose(1, 0, 2).reshape(128, 2 * D)
        ).astype(bfloat16)
        bkv_vec = np.concatenate([b1[ksl], b1[vsl]])
        in_maps.append({
            "xT": xTb[b],
            "w1q": np.ascontiguousarray(W1[:, qsl]).astype(bfloat16),
            "w1kv": np.ascontiguousarray(
                np.concatenate([W1[:, ksl], W1[:, vsl]], axis=1)).astype(bfloat16),
            "b1q": np.ascontiguousarray(b1[qsl].reshape(2, 128).T),
            "bkv": np.broadcast_to(bkv_vec[None, :], (128, 512)).astype(bfloat16),
            "w2c": w2c,
            "mask4": mask4,
            "ident": ident,
        })
    return in_maps


def kernel(x, W1, b1, W2, b2):
    nc = _get_nc()
    in_maps = make_core_inputs(x, W1, b1, W2, b2)
    kwargs = {}
    if TRACE:
        kwargs = {"trace": True, "tmpdir": TRACE_DIR}
    res = run_bass_kernel_spmd(nc, in_maps, list(range(8)), **kwargs)
    LAST_RESULTS[0] = res
    b2 = np.asarray(b2, dtype=np.float32)
    out = np.zeros((B, T, D), np.float32)
    for c in range(8):
        out[c // 4] += np.asarray(res.results[c]["out"], dtype=np.float32).T
    out += b2[None, None, :]
    return out

